# revision 16
# baseline (speedup 1.0000x reference)
"""Trainium2 Bass kernel for nn_DynamicPostionalBias.

Math: reference computes
    logits = einsum('nhid,hdj->nhij', query, rel_emb)        # [2,16,2048,4097]
    out[n,h,i,j] = logits[n,h,i, clip(j-i,-2047,2047)+2048]  # [2,16,2048,2048]
Since i,j in [0,2048), the clip is a no-op, so
    out[n,h,i,j] = sum_d q[n,h,i,d] * rel_emb[h,d, j-i+2048]
i.e. each output row i is a contiguous 2048-wide window of the logits row,
whose start shifts by -1 per row.

Strategy (8 NeuronCores): shard the 32 (n,h) pairs, 4 per core.  Host
pre-transposes q to [pair, d, i] so no on-device transpose is needed.
Per pair and per 128-row block, compute the needed logits window
[128, 2175] via bf16 matmuls ([64,128]^T @ [64,<=512] -> PSUM fp32),
copy+round PSUM->SBUF bf16 (split across Vector and Scalar engines;
GpSimd cannot access PSUM on TRN2), then store two blocks per DMA
whose SBUF source access pattern walks the per-partition shifted
windows: flat logical index of (row r, block b) is 127 + r*(nb*W-1) +
b*W, with the partition-crossing stride as the OUTERMOST dim and a
row-block-strided DRAM destination.  Input loads are staged (narrow
qT/E heads first) so matmuls start while the rest of the inputs
stream, keeping the DMA engines busy back-to-back from ~2us on.

The kernel is HBM-store-bound, so the output is stored as bf16 (the
problem's correctness gate is rel_err < 2e-2; bf16 rounding costs
~4e-3) and widened to fp32 on the host during the unshard/gather step.
This halves DMA traffic vs fp32 stores: ~207us -> ~105us per core.
"""

import sys

import numpy as np

for _p in ("/opt/trn_rl_repo", "/root/.axon_site/_ro/trn_rl_repo"):
    if _p not in sys.path:
        sys.path.append(_p)

import concourse.bass as bass
import concourse.mybir as mybir
import concourse.tile as tile
from concourse import bacc
from concourse.ap import AP
from concourse.bass_utils import run_bass_kernel_spmd

B, H, S, D = 2, 16, 2048, 64
J = 2 * S + 1  # 4097
G = 4          # (n,h) pairs per core
NB = S // 128  # 16 row blocks
W = S + 128    # 2176 sbuf tile width (2175 computed; even alloc)
CW = S + 127   # 2175 computed window width
N_CORES = 8

# bf16 throughout: 1 cycle/row on the PE regardless of moving-dim width,
# halves both the input-load and the (dominant) output-store DMA bytes.
MM_DTYPE = mybir.dt.bfloat16
OUT_DTYPE = mybir.dt.bfloat16

F32 = mybir.dt.float32

# chunk layout of the 2176-wide window (PSUM bank = 512 fp32; a matmul
# output must stay within one bank, but an engine copy may span banks).
# Only 2175 columns are needed; the 2176th is harmless (E col cs+2175 <= 4096).
# Each copy-chunk (c0, w, nmm) holds nmm 512-wide matmuls and one copy.
COPY_CHUNKS = [(0, 1024, 2), (1024, 1024, 2), (2048, 128, 1)]


def build_nc(mm_dtype=MM_DTYPE, out_dtype=OUT_DTYPE, nb=1, lp_bufs=3, reps=1,
             fold=True, load_eng_code="s", copy_map="vaa", store_engs="s",
             first_singles=2, staged_first=True):
    """nb = row-blocks batched per output DMA.

    E/qT for two consecutive pairs are folded into single 128-partition
    loads (pair gg in partitions 0-63, pair gg+1 in 64-127) for full DMA
    port coverage; matmuls address the upper half via base_partition=64.
    reps>1 re-runs the whole computation (for slope-based timing only).
    copy_map assigns each of the 3 PSUM->SBUF chunk copies to an engine
    (v=DVE, a=ACT; Pool cannot touch PSUM on TRN2); store_engs cycles
    output stores across the listed engines (s=SP/sync, a=ACT, v=DVE).
    first_singles: number of leading row blocks of pair 0 stored as
    single-block DMAs so the first store isn't gated on a full nb-group
    during the PE p-state ramp.
    load_eng_code: engine issuing input loads (s=SP/HWDGE, p=Pool/SWDGE).
    staged_first: split the first fold's qT/E loads into a narrow head
    (just the columns blocks 0-3 touch) + tail, so matmuls start ~3us
    earlier while the DMA engines keep streaming the remaining loads.
    """
    nc = bacc.Bacc("TRN2", target_bir_lowering=False, debug=False)
    qT = nc.declare_dram_parameter("qT", [G, D, S], mm_dtype, isOutput=False)
    E = nc.declare_dram_parameter("E", [G, D, J], mm_dtype, isOutput=False)
    out = nc.declare_dram_parameter("out", [G, S, S], out_dtype, isOutput=True)

    from contextlib import nullcontext

    eng_of = {
        "v": nc.vector,
        "a": nc.scalar,
        "p": nc.gpsimd,
        "s": nc.sync,
    }

    def copy_with(code, dst, src):
        if code == "a":
            nc.scalar.copy(dst, src)
        else:
            eng_of[code].tensor_copy(dst, src)

    def groups_for(g):
        """List of [t...] block groups stored per DMA for pair g."""
        start = 0
        groups = []
        if g == 0:
            for t in range(first_singles):
                groups.append([t])
            start = first_singles
        while start < NB:
            groups.append(list(range(start, min(start + nb, NB))))
            start += nb
        return groups

    with tile.TileContext(nc) as tc:
        with (
            tc.tile_pool(name="ep", bufs=2) as ep,
            tc.tile_pool(name="qp", bufs=2) as qp,
            tc.tile_pool(name="lp", bufs=lp_bufs) as lp,
            tc.tile_pool(name="pb", bufs=3, space=bass.MemorySpace.PSUM) as pb,
            tc.tile_pool(name="ps", bufs=2, space=bass.MemorySpace.PSUM) as psm,
        ):
            with (tc.For_i(0, reps, 1) if reps > 1 else nullcontext()):
                step = 2 if fold else 1
                load_eng = eng_of[load_eng_code]
                for gg in range(0, G, step):
                    pd = step * D
                    et = ep.tile([pd, J], mm_dtype, name=f"et{gg}", tag="et")
                    qt = qp.tile([pd, S], mm_dtype, name=f"qt{gg}", tag="qt")
                    if gg == 0 and staged_first:
                        # Narrow head loads: qT cols [0,256) and E cols
                        # [1793,4097) are all blocks 0-1 touch, so their
                        # matmuls can start ~3us before the full fold-0
                        # load would finish; the tails stream afterwards.
                        QH, EH = 256, 1793
                        load_eng.dma_start(
                            qt[:, :QH],
                            qT[gg : gg + step, :, :QH].rearrange(
                                "g d s -> (g d) s"),
                        )
                        # E head via Pool/SWDGE: its descriptor gen runs
                        # concurrently with the SP load's HWDGE, so the DMA
                        # engines see back-to-back transfers at startup.
                        nc.gpsimd.dma_start(
                            et[:, EH:],
                            E[gg : gg + step, :, EH:].rearrange(
                                "g d j -> (g d) j"),
                        )
                        load_eng.dma_start(
                            qt[:, QH:],
                            qT[gg : gg + step, :, QH:].rearrange(
                                "g d s -> (g d) s"),
                        )
                        load_eng.dma_start(
                            et[:, :EH],
                            E[gg : gg + step, :, :EH].rearrange(
                                "g d j -> (g d) j"),
                        )
                    else:
                        load_eng.dma_start(
                            et[:],
                            E[gg : gg + step].rearrange("g d j -> (g d) j"),
                        )
                        load_eng.dma_start(
                            qt[:],
                            qT[gg : gg + step].rearrange("g d s -> (g d) s"),
                        )
                    for g in range(gg, gg + step):
                        p0 = (g - gg) * D  # partition base within et/qt
                        for grp in groups_for(g):
                            nbg = len(grp)
                            t0 = grp[0]
                            lt = lp.tile([128, nbg * W], out_dtype,
                                         name=f"lt{g}_{t0}", tag="lt")
                            for b, t in enumerate(grp):
                                i0 = 128 * t
                                cs = S - i0 - 127  # window start column
                                for ci, (c0, w, nmm) in enumerate(COPY_CHUNKS):
                                    pool = pb if w > 512 else psm
                                    ps = pool.tile([128, w], F32,
                                                   name=f"ps{g}_{t}_{ci}",
                                                   tag="psb" if w > 512
                                                   else "pss")
                                    for m in range(nmm):
                                        mw = min(512, w - 512 * m)
                                        cm = cs + c0 + 512 * m
                                        nc.tensor.matmul(
                                            ps[:, 512 * m : 512 * m + mw],
                                            qt[p0 : p0 + D, i0 : i0 + 128],
                                            et[p0 : p0 + D, cm : cm + mw],
                                            start=True,
                                            stop=True,
                                        )
                                    lc = b * W + c0
                                    copy_with(
                                        copy_map[ci],
                                        lt[:, lc : lc + w],
                                        ps[:, :w],
                                    )
                            # out[g, 128*(t0+b)+r, j] = lt[r, b*W + 127-r + j]
                            # The partition-crossing flat stride must be the
                            # OUTERMOST src dim (as in the nb=1 form), so for
                            # nbg>1 iterate (r, b, j) on both sides and give
                            # the dst a strided (row-block) access pattern.
                            st_eng = eng_of[store_engs[t0 % len(store_engs)]]
                            if nbg == 1:
                                src = AP(
                                    tensor=lt.tensor,
                                    offset=lt.offset + 127,
                                    ap=[[W - 1, 128], [1, S]],
                                )
                                st_eng.dma_start(
                                    out[g, 128 * t0 : 128 * (t0 + 1), :], src
                                )
                            else:
                                src = AP(
                                    tensor=lt.tensor,
                                    offset=lt.offset + 127,
                                    ap=[[nbg * W - 1, 128], [W, nbg], [1, S]],
                                )
                                ofull = out[:, :, :]
                                dst = AP(
                                    tensor=ofull.tensor,
                                    offset=ofull.offset + g * S * S
                                    + 128 * t0 * S,
                                    ap=[[S, 128], [128 * S, nbg], [1, S]],
                                )
                                st_eng.dma_start(dst, src)
    nc.compile()
    return nc


_NC_CACHE = {}

NB_PER_DMA = 2
LP_BUFS = 3
COPY_MAP = "vaa"
STORE_ENGS = "s"
FIRST_SINGLES = 0


def _get_nc(mm_dtype=MM_DTYPE, out_dtype=OUT_DTYPE, nb=None, lp_bufs=None,
            copy_map=None, store_engs=None, first_singles=None, **bkw):
    nb = NB_PER_DMA if nb is None else nb
    lp_bufs = LP_BUFS if lp_bufs is None else lp_bufs
    copy_map = COPY_MAP if copy_map is None else copy_map
    store_engs = STORE_ENGS if store_engs is None else store_engs
    first_singles = FIRST_SINGLES if first_singles is None else first_singles
    key = (str(mm_dtype), str(out_dtype), nb, lp_bufs, copy_map, store_engs,
           first_singles, tuple(sorted(bkw.items())))
    if key not in _NC_CACHE:
        _NC_CACHE[key] = build_nc(mm_dtype, out_dtype, nb=nb, lp_bufs=lp_bufs,
                                  copy_map=copy_map, store_engs=store_engs,
                                  first_singles=first_singles, **bkw)
    return _NC_CACHE[key]


def make_in_maps(query, rel_emb, mm_dtype=MM_DTYPE):
    np_in = mybir.dt.np(mm_dtype)
    query = np.asarray(query, dtype=np.float32)
    rel_emb = np.asarray(rel_emb, dtype=np.float32).astype(np_in)
    # [B,H,S,D] -> [32, D, S], pair p = n*16 + h
    qTt = np.ascontiguousarray(
        query.reshape(B * H, S, D).transpose(0, 2, 1)
    ).astype(np_in)
    in_maps = []
    for k in range(N_CORES):
        h0 = 4 * (k % 4)
        in_maps.append(
            {
                "qT": qTt[4 * k : 4 * k + 4],
                "E": np.ascontiguousarray(rel_emb[h0 : h0 + 4]),
            }
        )
    return in_maps


def run_sharded(query, rel_emb, trace=False, mm_dtype=MM_DTYPE, **nc_kw):
    nc = _get_nc(mm_dtype, **nc_kw)
    in_maps = make_in_maps(query, rel_emb, mm_dtype)
    last_exc = None
    for attempt in range(3):
        if attempt:
            # transient device errors (e.g. NRT_EXEC_UNIT_UNRECOVERABLE)
            # have been observed to clear after a short cooldown
            import time

            time.sleep(20 * attempt)
        try:
            res = run_bass_kernel_spmd(
                nc, in_maps, list(range(N_CORES)), trace=trace
            )
            break
        except Exception as exc:  # noqa: BLE001 - retry transient device faults
            last_exc = exc
    else:
        raise last_exc
    full = np.empty((B * H, S, S), dtype=np.float32)
    for k in range(N_CORES):
        full[4 * k : 4 * k + 4] = np.asarray(
            res.results[k]["out"], dtype=np.float32
        )
    return full.reshape(B, H, S, S), res


def kernel(query, rel_emb, sequence_length=None):
    out, _ = run_sharded(query, rel_emb, trace=False)
    return out


# ---------------------------------------------------------------------------
# Timing harness (dev only): re-runnable sharded executable without donation,
# pipelined dispatch, null-kernel baseline subtraction.
# ---------------------------------------------------------------------------


def _prepare_exec(nc, in_maps, chain=1):
    import jax
    from jax.experimental.shard_map import shard_map
    from jax.sharding import Mesh, NamedSharding, PartitionSpec

    from concourse import bass2jax, mybir as mb

    bass2jax.install_neuronx_cc_hook()
    n_cores = len(in_maps)

    in_names, out_names, out_avals, zero_outs = [], [], [], []
    for alloc in nc.m.functions[0].allocations:
        if not isinstance(alloc, mb.MemoryLocationSet):
            continue
        name = alloc.memorylocations[0].name
        if alloc.kind == "ExternalInput":
            in_names.append(name)
        elif alloc.kind == "ExternalOutput":
            out_names.append(name)
            shape = tuple(alloc.tensor_shape)
            dtype = mb.dt.np(alloc.dtype)
            out_avals.append(jax.core.ShapedArray(shape, dtype))
            zero_outs.append(np.zeros(shape, dtype))
    partition_name = (
        nc.partition_id_tensor.name if nc.partition_id_tensor else None
    )
    if partition_name is not None and partition_name in in_names:
        in_names.remove(partition_name)
    n_params = len(in_names)
    in_names = in_names + out_names
    if partition_name is not None:
        in_names.append(partition_name)

    def _body(*args):
        operands = list(args)
        if partition_name is not None:
            operands.append(bass2jax.partition_id_tensor())
        for _ in range(chain):
            outs = bass2jax._bass_exec_p.bind(
                *operands,
                out_avals=tuple(out_avals),
                in_names=tuple(in_names),
                out_names=tuple(out_names),
                lowering_input_output_aliases=(),
                sim_require_finite=True,
                sim_require_nnan=True,
                nc=nc,
            )
        return tuple(outs)

    devices = jax.devices()[:n_cores]
    mesh = Mesh(np.asarray(devices), ("core",))
    spec = PartitionSpec("core")
    sharded = jax.jit(
        shard_map(
            _body,
            mesh=mesh,
            in_specs=(spec,) * (n_params + len(out_names)),
            out_specs=(spec,) * len(out_names),
            check_rep=False,
        ),
        keep_unused=True,
    )
    sh = NamedSharding(mesh, spec)
    per_core = [[np.asarray(m[name]) for name in in_names[:n_params]]
                for m in in_maps]
    args = [
        jax.device_put(
            np.concatenate([per_core[c][i] for c in range(n_cores)], axis=0), sh
        )
        for i in range(n_params)
    ]
    args += [
        jax.device_put(
            np.zeros((n_cores * z.shape[0], *z.shape[1:]), z.dtype), sh
        )
        for z in zero_outs
    ]
    return sharded, args


def build_null_nc():
    """Same I/O signature, near-zero work: for dispatch-overhead baseline."""
    nc = bacc.Bacc("TRN2", target_bir_lowering=False, debug=False)
    qT = nc.declare_dram_parameter("qT", [G, D, S], MM_DTYPE, isOutput=False)
    nc.declare_dram_parameter("E", [G, D, J], MM_DTYPE, isOutput=False)
    out = nc.declare_dram_parameter("out", [G, S, S], OUT_DTYPE, isOutput=True)
    with tile.TileContext(nc) as tc:
        with tc.tile_pool(name="p", bufs=1) as p:
            t = p.tile([64, 128], MM_DTYPE, name="t")
            nc.sync.dma_start(t[:], qT[0, :, :128])
            nc.sync.dma_start(out[0, :64, :128], t[:])
    nc.compile()
    return nc


def _time_callable(f, args, iters, reps=3):
    import time as _t

    import jax

    out = f(*args)
    jax.block_until_ready(out)
    best = float("inf")
    for _ in range(reps):
        t0 = _t.perf_counter()
        outs = [f(*args) for _ in range(iters)]
        jax.block_until_ready(outs)
        t1 = _t.perf_counter()
        best = min(best, (t1 - t0) / iters)
        del outs
    return best


def model_time_ns(mm_dtype=MM_DTYPE, **nc_kw):
    """Instruction-level cost-model (TimelineSim) estimate for one core."""
    from concourse.timeline_sim import TimelineSim

    return TimelineSim(_get_nc(mm_dtype, **nc_kw), trace=False).simulate()


def time_kernel(query, rel_emb, iters=6, mm_dtype=MM_DTYPE, rounds=4, **nc_kw):
    """Differential wall-clock: alternate (kernel, null-kernel with same I/O)
    pipelined batches; report median of per-round differences.  The axon
    dispatch overhead (~3 ms/call, noisy) mostly cancels; the cost-model
    estimate is typically the more trustworthy number."""
    in_maps = make_in_maps(query, rel_emb, mm_dtype)
    f, args = _prepare_exec(_get_nc(mm_dtype, **nc_kw), in_maps)
    f0, args0 = _prepare_exec(build_null_nc(), in_maps)
    tks, tns = [], []
    for _ in range(rounds):
        tks.append(_time_callable(f, args, iters, reps=1))
        tns.append(_time_callable(f0, args0, iters, reps=1))
    best = min(tks) - min(tns)
    print(f"  min kernel={min(tks)*1e6:.0f}us  min null={min(tns)*1e6:.0f}us  "
          f"diff-of-mins={best*1e6:.0f}us")
    return best * 1e9


# revision 17
# speedup vs baseline: 1.0017x; 1.0017x over previous
"""Trainium2 Bass kernel for nn_DynamicPostionalBias.

Math: reference computes
    logits = einsum('nhid,hdj->nhij', query, rel_emb)        # [2,16,2048,4097]
    out[n,h,i,j] = logits[n,h,i, clip(j-i,-2047,2047)+2048]  # [2,16,2048,2048]
Since i,j in [0,2048), the clip is a no-op, so
    out[n,h,i,j] = sum_d q[n,h,i,d] * rel_emb[h,d, j-i+2048]
i.e. each output row i is a contiguous 2048-wide window of the logits row,
whose start shifts by -1 per row.

Strategy (8 NeuronCores): shard the 32 (n,h) pairs, 4 per core.  Host
pre-transposes q to [pair, d, i] so no on-device transpose is needed.
Per pair and per 128-row block, compute the needed logits window
[128, 2175] via bf16 matmuls ([64,128]^T @ [64,<=512] -> PSUM fp32),
copy+round PSUM->SBUF bf16 (split across Vector and Scalar engines;
GpSimd cannot access PSUM on TRN2), then store two blocks per DMA
whose SBUF source access pattern walks the per-partition shifted
windows: flat logical index of (row r, block b) is 127 + r*(nb*W-1) +
b*W, with the partition-crossing stride as the OUTERMOST dim and a
row-block-strided DRAM destination.  Input loads are staged (narrow
qT/E heads first) so matmuls start while the rest of the inputs
stream, keeping the DMA engines busy back-to-back from ~2us on.

The kernel is HBM-store-bound, so the output is stored as bf16 (the
problem's correctness gate is rel_err < 2e-2; bf16 rounding costs
~4e-3) and widened to fp32 on the host during the unshard/gather step.
This halves DMA traffic vs fp32 stores: ~207us -> ~105us per core.
"""

import sys

import numpy as np

for _p in ("/opt/trn_rl_repo", "/root/.axon_site/_ro/trn_rl_repo"):
    if _p not in sys.path:
        sys.path.append(_p)

import concourse.bass as bass
import concourse.mybir as mybir
import concourse.tile as tile
from concourse import bacc
from concourse.ap import AP
from concourse.bass_utils import run_bass_kernel_spmd

B, H, S, D = 2, 16, 2048, 64
J = 2 * S + 1  # 4097
G = 4          # (n,h) pairs per core
NB = S // 128  # 16 row blocks
W = S + 128    # 2176 sbuf tile width (2175 computed; even alloc)
CW = S + 127   # 2175 computed window width
N_CORES = 8

# bf16 throughout: 1 cycle/row on the PE regardless of moving-dim width,
# halves both the input-load and the (dominant) output-store DMA bytes.
MM_DTYPE = mybir.dt.bfloat16
OUT_DTYPE = mybir.dt.bfloat16

F32 = mybir.dt.float32

# chunk layout of the 2176-wide window (PSUM bank = 512 fp32; a matmul
# output must stay within one bank, but an engine copy may span banks).
# Only 2175 columns are needed; the 2176th is harmless (E col cs+2175 <= 4096).
# Each copy-chunk (c0, w, nmm) holds nmm 512-wide matmuls and one copy.
COPY_CHUNKS = [(0, 1024, 2), (1024, 1024, 2), (2048, 128, 1)]


def build_nc(mm_dtype=MM_DTYPE, out_dtype=OUT_DTYPE, nb=1, lp_bufs=3, reps=1,
             fold=True, load_eng_code="s", copy_map="vaa", store_engs="s",
             first_singles=2, staged_first=True):
    """nb = row-blocks batched per output DMA.

    E/qT for two consecutive pairs are folded into single 128-partition
    loads (pair gg in partitions 0-63, pair gg+1 in 64-127) for full DMA
    port coverage; matmuls address the upper half via base_partition=64.
    reps>1 re-runs the whole computation (for slope-based timing only).
    copy_map assigns each of the 3 PSUM->SBUF chunk copies to an engine
    (v=DVE, a=ACT; Pool cannot touch PSUM on TRN2); store_engs cycles
    output stores across the listed engines (s=SP/sync, a=ACT, v=DVE).
    first_singles: number of leading row blocks of pair 0 stored as
    single-block DMAs so the first store isn't gated on a full nb-group
    during the PE p-state ramp.
    load_eng_code: engine issuing input loads (s=SP/HWDGE, p=Pool/SWDGE).
    staged_first: split the first fold's qT/E loads into a narrow head
    (just the columns blocks 0-3 touch) + tail, so matmuls start ~3us
    earlier while the DMA engines keep streaming the remaining loads.
    """
    nc = bacc.Bacc("TRN2", target_bir_lowering=False, debug=False)
    qT = nc.declare_dram_parameter("qT", [G, D, S], mm_dtype, isOutput=False)
    E = nc.declare_dram_parameter("E", [G, D, J], mm_dtype, isOutput=False)
    out = nc.declare_dram_parameter("out", [G, S, S], out_dtype, isOutput=True)

    from contextlib import nullcontext

    eng_of = {
        "v": nc.vector,
        "a": nc.scalar,
        "p": nc.gpsimd,
        "s": nc.sync,
    }

    def copy_with(code, dst, src):
        if code == "a":
            nc.scalar.copy(dst, src)
        else:
            eng_of[code].tensor_copy(dst, src)

    def groups_for(g):
        """List of [t...] block groups stored per DMA for pair g."""
        start = 0
        groups = []
        if g == 0:
            for t in range(first_singles):
                groups.append([t])
            start = first_singles
        while start < NB:
            groups.append(list(range(start, min(start + nb, NB))))
            start += nb
        return groups

    with tile.TileContext(nc) as tc:
        with (
            tc.tile_pool(name="ep", bufs=2) as ep,
            tc.tile_pool(name="qp", bufs=2) as qp,
            tc.tile_pool(name="lp", bufs=lp_bufs) as lp,
            tc.tile_pool(name="pb", bufs=3, space=bass.MemorySpace.PSUM) as pb,
            tc.tile_pool(name="ps", bufs=2, space=bass.MemorySpace.PSUM) as psm,
        ):
            with (tc.For_i(0, reps, 1) if reps > 1 else nullcontext()):
                step = 2 if fold else 1
                load_eng = eng_of[load_eng_code]
                for gg in range(0, G, step):
                    pd = step * D
                    et = ep.tile([pd, J], mm_dtype, name=f"et{gg}", tag="et")
                    qt = qp.tile([pd, S], mm_dtype, name=f"qt{gg}", tag="qt")
                    if gg == 0 and staged_first:
                        # Narrow head loads: qT cols [0,256) and E cols
                        # [1793,4097) are all blocks 0-1 touch, so their
                        # matmuls can start ~3us before the full fold-0
                        # load would finish; the tails stream afterwards.
                        QH, EH = 512, 1793
                        load_eng.dma_start(
                            qt[:, :QH],
                            qT[gg : gg + step, :, :QH].rearrange(
                                "g d s -> (g d) s"),
                        )
                        # E head via Pool/SWDGE: its descriptor gen runs
                        # concurrently with the SP load's HWDGE, so the DMA
                        # engines see back-to-back transfers at startup.
                        nc.gpsimd.dma_start(
                            et[:, EH:],
                            E[gg : gg + step, :, EH:].rearrange(
                                "g d j -> (g d) j"),
                        )
                        load_eng.dma_start(
                            qt[:, QH:],
                            qT[gg : gg + step, :, QH:].rearrange(
                                "g d s -> (g d) s"),
                        )
                        load_eng.dma_start(
                            et[:, :EH],
                            E[gg : gg + step, :, :EH].rearrange(
                                "g d j -> (g d) j"),
                        )
                    else:
                        load_eng.dma_start(
                            et[:],
                            E[gg : gg + step].rearrange("g d j -> (g d) j"),
                        )
                        load_eng.dma_start(
                            qt[:],
                            qT[gg : gg + step].rearrange("g d s -> (g d) s"),
                        )
                    for g in range(gg, gg + step):
                        p0 = (g - gg) * D  # partition base within et/qt
                        for grp in groups_for(g):
                            nbg = len(grp)
                            t0 = grp[0]
                            lt = lp.tile([128, nbg * W], out_dtype,
                                         name=f"lt{g}_{t0}", tag="lt")
                            for b, t in enumerate(grp):
                                i0 = 128 * t
                                cs = S - i0 - 127  # window start column
                                for ci, (c0, w, nmm) in enumerate(COPY_CHUNKS):
                                    pool = pb if w > 512 else psm
                                    ps = pool.tile([128, w], F32,
                                                   name=f"ps{g}_{t}_{ci}",
                                                   tag="psb" if w > 512
                                                   else "pss")
                                    for m in range(nmm):
                                        mw = min(512, w - 512 * m)
                                        cm = cs + c0 + 512 * m
                                        nc.tensor.matmul(
                                            ps[:, 512 * m : 512 * m + mw],
                                            qt[p0 : p0 + D, i0 : i0 + 128],
                                            et[p0 : p0 + D, cm : cm + mw],
                                            start=True,
                                            stop=True,
                                        )
                                    lc = b * W + c0
                                    copy_with(
                                        copy_map[ci],
                                        lt[:, lc : lc + w],
                                        ps[:, :w],
                                    )
                            # out[g, 128*(t0+b)+r, j] = lt[r, b*W + 127-r + j]
                            # The partition-crossing flat stride must be the
                            # OUTERMOST src dim (as in the nb=1 form), so for
                            # nbg>1 iterate (r, b, j) on both sides and give
                            # the dst a strided (row-block) access pattern.
                            st_eng = eng_of[store_engs[t0 % len(store_engs)]]
                            if nbg == 1:
                                src = AP(
                                    tensor=lt.tensor,
                                    offset=lt.offset + 127,
                                    ap=[[W - 1, 128], [1, S]],
                                )
                                st_eng.dma_start(
                                    out[g, 128 * t0 : 128 * (t0 + 1), :], src
                                )
                            else:
                                src = AP(
                                    tensor=lt.tensor,
                                    offset=lt.offset + 127,
                                    ap=[[nbg * W - 1, 128], [W, nbg], [1, S]],
                                )
                                ofull = out[:, :, :]
                                dst = AP(
                                    tensor=ofull.tensor,
                                    offset=ofull.offset + g * S * S
                                    + 128 * t0 * S,
                                    ap=[[S, 128], [128 * S, nbg], [1, S]],
                                )
                                st_eng.dma_start(dst, src)
    nc.compile()
    return nc


_NC_CACHE = {}

NB_PER_DMA = 2
LP_BUFS = 3
COPY_MAP = "vaa"
STORE_ENGS = "s"
FIRST_SINGLES = 0


def _get_nc(mm_dtype=MM_DTYPE, out_dtype=OUT_DTYPE, nb=None, lp_bufs=None,
            copy_map=None, store_engs=None, first_singles=None, **bkw):
    nb = NB_PER_DMA if nb is None else nb
    lp_bufs = LP_BUFS if lp_bufs is None else lp_bufs
    copy_map = COPY_MAP if copy_map is None else copy_map
    store_engs = STORE_ENGS if store_engs is None else store_engs
    first_singles = FIRST_SINGLES if first_singles is None else first_singles
    key = (str(mm_dtype), str(out_dtype), nb, lp_bufs, copy_map, store_engs,
           first_singles, tuple(sorted(bkw.items())))
    if key not in _NC_CACHE:
        _NC_CACHE[key] = build_nc(mm_dtype, out_dtype, nb=nb, lp_bufs=lp_bufs,
                                  copy_map=copy_map, store_engs=store_engs,
                                  first_singles=first_singles, **bkw)
    return _NC_CACHE[key]


def make_in_maps(query, rel_emb, mm_dtype=MM_DTYPE):
    np_in = mybir.dt.np(mm_dtype)
    query = np.asarray(query, dtype=np.float32)
    rel_emb = np.asarray(rel_emb, dtype=np.float32).astype(np_in)
    # [B,H,S,D] -> [32, D, S], pair p = n*16 + h
    qTt = np.ascontiguousarray(
        query.reshape(B * H, S, D).transpose(0, 2, 1)
    ).astype(np_in)
    in_maps = []
    for k in range(N_CORES):
        h0 = 4 * (k % 4)
        in_maps.append(
            {
                "qT": qTt[4 * k : 4 * k + 4],
                "E": np.ascontiguousarray(rel_emb[h0 : h0 + 4]),
            }
        )
    return in_maps


def run_sharded(query, rel_emb, trace=False, mm_dtype=MM_DTYPE, **nc_kw):
    nc = _get_nc(mm_dtype, **nc_kw)
    in_maps = make_in_maps(query, rel_emb, mm_dtype)
    last_exc = None
    for attempt in range(3):
        if attempt:
            # transient device errors (e.g. NRT_EXEC_UNIT_UNRECOVERABLE)
            # have been observed to clear after a short cooldown
            import time

            time.sleep(20 * attempt)
        try:
            res = run_bass_kernel_spmd(
                nc, in_maps, list(range(N_CORES)), trace=trace
            )
            break
        except Exception as exc:  # noqa: BLE001 - retry transient device faults
            last_exc = exc
    else:
        raise last_exc
    full = np.empty((B * H, S, S), dtype=np.float32)
    for k in range(N_CORES):
        full[4 * k : 4 * k + 4] = np.asarray(
            res.results[k]["out"], dtype=np.float32
        )
    return full.reshape(B, H, S, S), res


def kernel(query, rel_emb, sequence_length=None):
    out, _ = run_sharded(query, rel_emb, trace=False)
    return out


# ---------------------------------------------------------------------------
# Timing harness (dev only): re-runnable sharded executable without donation,
# pipelined dispatch, null-kernel baseline subtraction.
# ---------------------------------------------------------------------------


def _prepare_exec(nc, in_maps, chain=1):
    import jax
    from jax.experimental.shard_map import shard_map
    from jax.sharding import Mesh, NamedSharding, PartitionSpec

    from concourse import bass2jax, mybir as mb

    bass2jax.install_neuronx_cc_hook()
    n_cores = len(in_maps)

    in_names, out_names, out_avals, zero_outs = [], [], [], []
    for alloc in nc.m.functions[0].allocations:
        if not isinstance(alloc, mb.MemoryLocationSet):
            continue
        name = alloc.memorylocations[0].name
        if alloc.kind == "ExternalInput":
            in_names.append(name)
        elif alloc.kind == "ExternalOutput":
            out_names.append(name)
            shape = tuple(alloc.tensor_shape)
            dtype = mb.dt.np(alloc.dtype)
            out_avals.append(jax.core.ShapedArray(shape, dtype))
            zero_outs.append(np.zeros(shape, dtype))
    partition_name = (
        nc.partition_id_tensor.name if nc.partition_id_tensor else None
    )
    if partition_name is not None and partition_name in in_names:
        in_names.remove(partition_name)
    n_params = len(in_names)
    in_names = in_names + out_names
    if partition_name is not None:
        in_names.append(partition_name)

    def _body(*args):
        operands = list(args)
        if partition_name is not None:
            operands.append(bass2jax.partition_id_tensor())
        for _ in range(chain):
            outs = bass2jax._bass_exec_p.bind(
                *operands,
                out_avals=tuple(out_avals),
                in_names=tuple(in_names),
                out_names=tuple(out_names),
                lowering_input_output_aliases=(),
                sim_require_finite=True,
                sim_require_nnan=True,
                nc=nc,
            )
        return tuple(outs)

    devices = jax.devices()[:n_cores]
    mesh = Mesh(np.asarray(devices), ("core",))
    spec = PartitionSpec("core")
    sharded = jax.jit(
        shard_map(
            _body,
            mesh=mesh,
            in_specs=(spec,) * (n_params + len(out_names)),
            out_specs=(spec,) * len(out_names),
            check_rep=False,
        ),
        keep_unused=True,
    )
    sh = NamedSharding(mesh, spec)
    per_core = [[np.asarray(m[name]) for name in in_names[:n_params]]
                for m in in_maps]
    args = [
        jax.device_put(
            np.concatenate([per_core[c][i] for c in range(n_cores)], axis=0), sh
        )
        for i in range(n_params)
    ]
    args += [
        jax.device_put(
            np.zeros((n_cores * z.shape[0], *z.shape[1:]), z.dtype), sh
        )
        for z in zero_outs
    ]
    return sharded, args


def build_null_nc():
    """Same I/O signature, near-zero work: for dispatch-overhead baseline."""
    nc = bacc.Bacc("TRN2", target_bir_lowering=False, debug=False)
    qT = nc.declare_dram_parameter("qT", [G, D, S], MM_DTYPE, isOutput=False)
    nc.declare_dram_parameter("E", [G, D, J], MM_DTYPE, isOutput=False)
    out = nc.declare_dram_parameter("out", [G, S, S], OUT_DTYPE, isOutput=True)
    with tile.TileContext(nc) as tc:
        with tc.tile_pool(name="p", bufs=1) as p:
            t = p.tile([64, 128], MM_DTYPE, name="t")
            nc.sync.dma_start(t[:], qT[0, :, :128])
            nc.sync.dma_start(out[0, :64, :128], t[:])
    nc.compile()
    return nc


def _time_callable(f, args, iters, reps=3):
    import time as _t

    import jax

    out = f(*args)
    jax.block_until_ready(out)
    best = float("inf")
    for _ in range(reps):
        t0 = _t.perf_counter()
        outs = [f(*args) for _ in range(iters)]
        jax.block_until_ready(outs)
        t1 = _t.perf_counter()
        best = min(best, (t1 - t0) / iters)
        del outs
    return best


def model_time_ns(mm_dtype=MM_DTYPE, **nc_kw):
    """Instruction-level cost-model (TimelineSim) estimate for one core."""
    from concourse.timeline_sim import TimelineSim

    return TimelineSim(_get_nc(mm_dtype, **nc_kw), trace=False).simulate()


def time_kernel(query, rel_emb, iters=6, mm_dtype=MM_DTYPE, rounds=4, **nc_kw):
    """Differential wall-clock: alternate (kernel, null-kernel with same I/O)
    pipelined batches; report median of per-round differences.  The axon
    dispatch overhead (~3 ms/call, noisy) mostly cancels; the cost-model
    estimate is typically the more trustworthy number."""
    in_maps = make_in_maps(query, rel_emb, mm_dtype)
    f, args = _prepare_exec(_get_nc(mm_dtype, **nc_kw), in_maps)
    f0, args0 = _prepare_exec(build_null_nc(), in_maps)
    tks, tns = [], []
    for _ in range(rounds):
        tks.append(_time_callable(f, args, iters, reps=1))
        tns.append(_time_callable(f0, args0, iters, reps=1))
    best = min(tks) - min(tns)
    print(f"  min kernel={min(tks)*1e6:.0f}us  min null={min(tns)*1e6:.0f}us  "
          f"diff-of-mins={best*1e6:.0f}us")
    return best * 1e9


# revision 20
# speedup vs baseline: 1.0021x; 1.0004x over previous
"""Trainium2 Bass kernel for nn_DynamicPostionalBias.

Math: reference computes
    logits = einsum('nhid,hdj->nhij', query, rel_emb)        # [2,16,2048,4097]
    out[n,h,i,j] = logits[n,h,i, clip(j-i,-2047,2047)+2048]  # [2,16,2048,2048]
Since i,j in [0,2048), the clip is a no-op, so
    out[n,h,i,j] = sum_d q[n,h,i,d] * rel_emb[h,d, j-i+2048]
i.e. each output row i is a contiguous 2048-wide window of the logits row,
whose start shifts by -1 per row.

Strategy (8 NeuronCores): shard the 32 (n,h) pairs, 4 per core.  Host
pre-transposes q to [pair, d, i] so no on-device transpose is needed.
Per pair and per 128-row block, compute the needed logits window
[128, 2175] via bf16 matmuls ([64,128]^T @ [64,<=512] -> PSUM fp32),
copy+round PSUM->SBUF bf16 (split across Vector and Scalar engines;
GpSimd cannot access PSUM on TRN2), then store two blocks per DMA
whose SBUF source access pattern walks the per-partition shifted
windows: flat logical index of (row r, block b) is 127 + r*(nb*W-1) +
b*W, with the partition-crossing stride as the OUTERMOST dim and a
row-block-strided DRAM destination.  Input loads are staged (narrow
qT/E heads first) so matmuls start while the rest of the inputs
stream, keeping the DMA engines busy back-to-back from ~2us on.

The kernel is HBM-store-bound, so the output is stored as bf16 (the
problem's correctness gate is rel_err < 2e-2; bf16 rounding costs
~4e-3) and widened to fp32 on the host during the unshard/gather step.
This halves DMA traffic vs fp32 stores: ~207us -> ~105us per core.
"""

import sys

import numpy as np

for _p in ("/opt/trn_rl_repo", "/root/.axon_site/_ro/trn_rl_repo"):
    if _p not in sys.path:
        sys.path.append(_p)

import concourse.bass as bass
import concourse.mybir as mybir
import concourse.tile as tile
from concourse import bacc
from concourse.ap import AP
from concourse.bass_utils import run_bass_kernel_spmd

B, H, S, D = 2, 16, 2048, 64
J = 2 * S + 1  # 4097
G = 4          # (n,h) pairs per core
NB = S // 128  # 16 row blocks
W = S + 128    # 2176 sbuf tile width (2175 computed; even alloc)
CW = S + 127   # 2175 computed window width
N_CORES = 8

# bf16 throughout: 1 cycle/row on the PE regardless of moving-dim width,
# halves both the input-load and the (dominant) output-store DMA bytes.
MM_DTYPE = mybir.dt.bfloat16
OUT_DTYPE = mybir.dt.bfloat16

F32 = mybir.dt.float32

# chunk layout of the 2176-wide window (PSUM bank = 512 fp32; a matmul
# output must stay within one bank, but an engine copy may span banks).
# Only 2175 columns are needed; the 2176th is harmless (E col cs+2175 <= 4096).
# Each copy-chunk (c0, w, nmm) holds nmm 512-wide matmuls and one copy.
COPY_CHUNKS = [(0, 1024, 2), (1024, 1024, 2), (2048, 128, 1)]


def build_nc(mm_dtype=MM_DTYPE, out_dtype=OUT_DTYPE, nb=1, lp_bufs=3, reps=1,
             fold=True, load_eng_code="s", copy_map="vaa", store_engs="s",
             first_singles=2, staged_first=True):
    """nb = row-blocks batched per output DMA.

    E/qT for two consecutive pairs are folded into single 128-partition
    loads (pair gg in partitions 0-63, pair gg+1 in 64-127) for full DMA
    port coverage; matmuls address the upper half via base_partition=64.
    reps>1 re-runs the whole computation (for slope-based timing only).
    copy_map assigns each of the 3 PSUM->SBUF chunk copies to an engine
    (v=DVE, a=ACT; Pool cannot touch PSUM on TRN2); store_engs cycles
    output stores across the listed engines (s=SP/sync, a=ACT, v=DVE).
    first_singles: number of leading row blocks of pair 0 stored as
    single-block DMAs so the first store isn't gated on a full nb-group
    during the PE p-state ramp.
    load_eng_code: engine issuing input loads (s=SP/HWDGE, p=Pool/SWDGE).
    staged_first: split the first fold's qT/E loads into a narrow head
    (just the columns blocks 0-3 touch) + tail, so matmuls start ~3us
    earlier while the DMA engines keep streaming the remaining loads.
    """
    nc = bacc.Bacc("TRN2", target_bir_lowering=False, debug=False)
    qT = nc.declare_dram_parameter("qT", [G, D, S], mm_dtype, isOutput=False)
    E = nc.declare_dram_parameter("E", [G, D, J], mm_dtype, isOutput=False)
    out = nc.declare_dram_parameter("out", [G, S, S], out_dtype, isOutput=True)

    from contextlib import nullcontext

    eng_of = {
        "v": nc.vector,
        "a": nc.scalar,
        "p": nc.gpsimd,
        "s": nc.sync,
    }

    def copy_with(code, dst, src):
        if code == "a":
            nc.scalar.copy(dst, src)
        else:
            eng_of[code].tensor_copy(dst, src)

    def groups_for(g):
        """List of [t...] block groups stored per DMA for pair g."""
        start = 0
        groups = []
        if g == 0:
            for t in range(first_singles):
                groups.append([t])
            start = first_singles
        while start < NB:
            groups.append(list(range(start, min(start + nb, NB))))
            start += nb
        return groups

    with tile.TileContext(nc) as tc:
        with (
            tc.tile_pool(name="ep", bufs=2) as ep,
            tc.tile_pool(name="qp", bufs=2) as qp,
            tc.tile_pool(name="lp", bufs=lp_bufs) as lp,
            tc.tile_pool(name="pb", bufs=3, space=bass.MemorySpace.PSUM) as pb,
            tc.tile_pool(name="ps", bufs=2, space=bass.MemorySpace.PSUM) as psm,
        ):
            with (tc.For_i(0, reps, 1) if reps > 1 else nullcontext()):
                step = 2 if fold else 1
                load_eng = eng_of[load_eng_code]
                for gg in range(0, G, step):
                    pd = step * D
                    et = ep.tile([pd, J], mm_dtype, name=f"et{gg}", tag="et")
                    qt = qp.tile([pd, S], mm_dtype, name=f"qt{gg}", tag="qt")
                    if gg == 0 and staged_first:
                        # Narrow head loads: qT cols [0,256) and E cols
                        # [1793,4097) are all blocks 0-1 touch, so their
                        # matmuls can start ~3us before the full fold-0
                        # load would finish; the tails stream afterwards.
                        QH, EH = 576, 1793
                        load_eng.dma_start(
                            qt[:, :QH],
                            qT[gg : gg + step, :, :QH].rearrange(
                                "g d s -> (g d) s"),
                        )
                        # E head via Pool/SWDGE: its descriptor gen runs
                        # concurrently with the SP load's HWDGE, so the DMA
                        # engines see back-to-back transfers at startup.
                        nc.gpsimd.dma_start(
                            et[:, EH:],
                            E[gg : gg + step, :, EH:].rearrange(
                                "g d j -> (g d) j"),
                        )
                        load_eng.dma_start(
                            qt[:, QH:],
                            qT[gg : gg + step, :, QH:].rearrange(
                                "g d s -> (g d) s"),
                        )
                        load_eng.dma_start(
                            et[:, :EH],
                            E[gg : gg + step, :, :EH].rearrange(
                                "g d j -> (g d) j"),
                        )
                    else:
                        load_eng.dma_start(
                            et[:],
                            E[gg : gg + step].rearrange("g d j -> (g d) j"),
                        )
                        load_eng.dma_start(
                            qt[:],
                            qT[gg : gg + step].rearrange("g d s -> (g d) s"),
                        )
                    for g in range(gg, gg + step):
                        p0 = (g - gg) * D  # partition base within et/qt
                        for gi, grp in enumerate(groups_for(g)):
                            nbg = len(grp)
                            t0 = grp[0]
                            lt = lp.tile([128, nbg * W], out_dtype,
                                         name=f"lt{g}_{t0}", tag="lt")
                            for b, t in enumerate(grp):
                                i0 = 128 * t
                                cs = S - i0 - 127  # window start column
                                for ci, (c0, w, nmm) in enumerate(COPY_CHUNKS):
                                    pool = pb if w > 512 else psm
                                    ps = pool.tile([128, w], F32,
                                                   name=f"ps{g}_{t}_{ci}",
                                                   tag="psb" if w > 512
                                                   else "pss")
                                    for m in range(nmm):
                                        mw = min(512, w - 512 * m)
                                        cm = cs + c0 + 512 * m
                                        nc.tensor.matmul(
                                            ps[:, 512 * m : 512 * m + mw],
                                            qt[p0 : p0 + D, i0 : i0 + 128],
                                            et[p0 : p0 + D, cm : cm + mw],
                                            start=True,
                                            stop=True,
                                        )
                                    lc = b * W + c0
                                    copy_with(
                                        copy_map[ci],
                                        lt[:, lc : lc + w],
                                        ps[:, :w],
                                    )
                            # out[g, 128*(t0+b)+r, j] = lt[r, b*W + 127-r + j]
                            # The partition-crossing flat stride must be the
                            # OUTERMOST src dim (as in the nb=1 form), so for
                            # nbg>1 iterate (r, b, j) on both sides and give
                            # the dst a strided (row-block) access pattern.
                            st_eng = eng_of[store_engs[gi % len(store_engs)]]
                            if nbg == 1:
                                src = AP(
                                    tensor=lt.tensor,
                                    offset=lt.offset + 127,
                                    ap=[[W - 1, 128], [1, S]],
                                )
                                st_eng.dma_start(
                                    out[g, 128 * t0 : 128 * (t0 + 1), :], src
                                )
                            else:
                                src = AP(
                                    tensor=lt.tensor,
                                    offset=lt.offset + 127,
                                    ap=[[nbg * W - 1, 128], [W, nbg], [1, S]],
                                )
                                ofull = out[:, :, :]
                                dst = AP(
                                    tensor=ofull.tensor,
                                    offset=ofull.offset + g * S * S
                                    + 128 * t0 * S,
                                    ap=[[S, 128], [128 * S, nbg], [1, S]],
                                )
                                st_eng.dma_start(dst, src)
    nc.compile()
    return nc


_NC_CACHE = {}

NB_PER_DMA = 2
LP_BUFS = 3
COPY_MAP = "vaa"
STORE_ENGS = "s"
FIRST_SINGLES = 0


def _get_nc(mm_dtype=MM_DTYPE, out_dtype=OUT_DTYPE, nb=None, lp_bufs=None,
            copy_map=None, store_engs=None, first_singles=None, **bkw):
    nb = NB_PER_DMA if nb is None else nb
    lp_bufs = LP_BUFS if lp_bufs is None else lp_bufs
    copy_map = COPY_MAP if copy_map is None else copy_map
    store_engs = STORE_ENGS if store_engs is None else store_engs
    first_singles = FIRST_SINGLES if first_singles is None else first_singles
    key = (str(mm_dtype), str(out_dtype), nb, lp_bufs, copy_map, store_engs,
           first_singles, tuple(sorted(bkw.items())))
    if key not in _NC_CACHE:
        _NC_CACHE[key] = build_nc(mm_dtype, out_dtype, nb=nb, lp_bufs=lp_bufs,
                                  copy_map=copy_map, store_engs=store_engs,
                                  first_singles=first_singles, **bkw)
    return _NC_CACHE[key]


def make_in_maps(query, rel_emb, mm_dtype=MM_DTYPE):
    np_in = mybir.dt.np(mm_dtype)
    query = np.asarray(query, dtype=np.float32)
    rel_emb = np.asarray(rel_emb, dtype=np.float32).astype(np_in)
    # [B,H,S,D] -> [32, D, S], pair p = n*16 + h
    qTt = np.ascontiguousarray(
        query.reshape(B * H, S, D).transpose(0, 2, 1)
    ).astype(np_in)
    in_maps = []
    for k in range(N_CORES):
        h0 = 4 * (k % 4)
        in_maps.append(
            {
                "qT": qTt[4 * k : 4 * k + 4],
                "E": np.ascontiguousarray(rel_emb[h0 : h0 + 4]),
            }
        )
    return in_maps


def run_sharded(query, rel_emb, trace=False, mm_dtype=MM_DTYPE, **nc_kw):
    nc = _get_nc(mm_dtype, **nc_kw)
    in_maps = make_in_maps(query, rel_emb, mm_dtype)
    last_exc = None
    for attempt in range(3):
        if attempt:
            # transient device errors (e.g. NRT_EXEC_UNIT_UNRECOVERABLE)
            # have been observed to clear after a short cooldown
            import time

            time.sleep(20 * attempt)
        try:
            res = run_bass_kernel_spmd(
                nc, in_maps, list(range(N_CORES)), trace=trace
            )
            break
        except Exception as exc:  # noqa: BLE001 - retry transient device faults
            last_exc = exc
    else:
        raise last_exc
    full = np.empty((B * H, S, S), dtype=np.float32)
    for k in range(N_CORES):
        full[4 * k : 4 * k + 4] = np.asarray(
            res.results[k]["out"], dtype=np.float32
        )
    return full.reshape(B, H, S, S), res


def kernel(query, rel_emb, sequence_length=None):
    out, _ = run_sharded(query, rel_emb, trace=False)
    return out


# ---------------------------------------------------------------------------
# Timing harness (dev only): re-runnable sharded executable without donation,
# pipelined dispatch, null-kernel baseline subtraction.
# ---------------------------------------------------------------------------


def _prepare_exec(nc, in_maps, chain=1):
    import jax
    from jax.experimental.shard_map import shard_map
    from jax.sharding import Mesh, NamedSharding, PartitionSpec

    from concourse import bass2jax, mybir as mb

    bass2jax.install_neuronx_cc_hook()
    n_cores = len(in_maps)

    in_names, out_names, out_avals, zero_outs = [], [], [], []
    for alloc in nc.m.functions[0].allocations:
        if not isinstance(alloc, mb.MemoryLocationSet):
            continue
        name = alloc.memorylocations[0].name
        if alloc.kind == "ExternalInput":
            in_names.append(name)
        elif alloc.kind == "ExternalOutput":
            out_names.append(name)
            shape = tuple(alloc.tensor_shape)
            dtype = mb.dt.np(alloc.dtype)
            out_avals.append(jax.core.ShapedArray(shape, dtype))
            zero_outs.append(np.zeros(shape, dtype))
    partition_name = (
        nc.partition_id_tensor.name if nc.partition_id_tensor else None
    )
    if partition_name is not None and partition_name in in_names:
        in_names.remove(partition_name)
    n_params = len(in_names)
    in_names = in_names + out_names
    if partition_name is not None:
        in_names.append(partition_name)

    def _body(*args):
        operands = list(args)
        if partition_name is not None:
            operands.append(bass2jax.partition_id_tensor())
        for _ in range(chain):
            outs = bass2jax._bass_exec_p.bind(
                *operands,
                out_avals=tuple(out_avals),
                in_names=tuple(in_names),
                out_names=tuple(out_names),
                lowering_input_output_aliases=(),
                sim_require_finite=True,
                sim_require_nnan=True,
                nc=nc,
            )
        return tuple(outs)

    devices = jax.devices()[:n_cores]
    mesh = Mesh(np.asarray(devices), ("core",))
    spec = PartitionSpec("core")
    sharded = jax.jit(
        shard_map(
            _body,
            mesh=mesh,
            in_specs=(spec,) * (n_params + len(out_names)),
            out_specs=(spec,) * len(out_names),
            check_rep=False,
        ),
        keep_unused=True,
    )
    sh = NamedSharding(mesh, spec)
    per_core = [[np.asarray(m[name]) for name in in_names[:n_params]]
                for m in in_maps]
    args = [
        jax.device_put(
            np.concatenate([per_core[c][i] for c in range(n_cores)], axis=0), sh
        )
        for i in range(n_params)
    ]
    args += [
        jax.device_put(
            np.zeros((n_cores * z.shape[0], *z.shape[1:]), z.dtype), sh
        )
        for z in zero_outs
    ]
    return sharded, args


def build_null_nc():
    """Same I/O signature, near-zero work: for dispatch-overhead baseline."""
    nc = bacc.Bacc("TRN2", target_bir_lowering=False, debug=False)
    qT = nc.declare_dram_parameter("qT", [G, D, S], MM_DTYPE, isOutput=False)
    nc.declare_dram_parameter("E", [G, D, J], MM_DTYPE, isOutput=False)
    out = nc.declare_dram_parameter("out", [G, S, S], OUT_DTYPE, isOutput=True)
    with tile.TileContext(nc) as tc:
        with tc.tile_pool(name="p", bufs=1) as p:
            t = p.tile([64, 128], MM_DTYPE, name="t")
            nc.sync.dma_start(t[:], qT[0, :, :128])
            nc.sync.dma_start(out[0, :64, :128], t[:])
    nc.compile()
    return nc


def _time_callable(f, args, iters, reps=3):
    import time as _t

    import jax

    out = f(*args)
    jax.block_until_ready(out)
    best = float("inf")
    for _ in range(reps):
        t0 = _t.perf_counter()
        outs = [f(*args) for _ in range(iters)]
        jax.block_until_ready(outs)
        t1 = _t.perf_counter()
        best = min(best, (t1 - t0) / iters)
        del outs
    return best


def model_time_ns(mm_dtype=MM_DTYPE, **nc_kw):
    """Instruction-level cost-model (TimelineSim) estimate for one core."""
    from concourse.timeline_sim import TimelineSim

    return TimelineSim(_get_nc(mm_dtype, **nc_kw), trace=False).simulate()


def time_kernel(query, rel_emb, iters=6, mm_dtype=MM_DTYPE, rounds=4, **nc_kw):
    """Differential wall-clock: alternate (kernel, null-kernel with same I/O)
    pipelined batches; report median of per-round differences.  The axon
    dispatch overhead (~3 ms/call, noisy) mostly cancels; the cost-model
    estimate is typically the more trustworthy number."""
    in_maps = make_in_maps(query, rel_emb, mm_dtype)
    f, args = _prepare_exec(_get_nc(mm_dtype, **nc_kw), in_maps)
    f0, args0 = _prepare_exec(build_null_nc(), in_maps)
    tks, tns = [], []
    for _ in range(rounds):
        tks.append(_time_callable(f, args, iters, reps=1))
        tns.append(_time_callable(f0, args0, iters, reps=1))
    best = min(tks) - min(tns)
    print(f"  min kernel={min(tks)*1e6:.0f}us  min null={min(tns)*1e6:.0f}us  "
          f"diff-of-mins={best*1e6:.0f}us")
    return best * 1e9
